# revision 17
# baseline (speedup 1.0000x reference)
"""Trainium2 Bass kernel for nn_CameraEstimator.

For each batch item b:
    camera[b] = einsum('chw,c->hw', x[b], W)          (C=256 contraction)
    out[b]    = nearest-rotation(camera[b])           (SVD u@vh + det reflection fix)

Pipeline (per core, B split 8 ways):
  * Host packs x into a transposed split-fp16 layout: for each 128-batch tile,
    partition k holds the (c,e)-flattened entries ce = 128j + k (j = 0..17) for
    128 batch items, as fp16 hi and fp16 lo (hi = fp16(x), lo = fp16(x - hi)).
    Same total HBM bytes as fp32 x; enables direct PE contraction with zero
    on-device transposes/casts.
  * Device: per tile, 36 accumulating matmuls (18 hi + 18 lo chunks) against a
    masked-W operand -> camera [128, 9hi+9lo] in PSUM; one DVE add folds them.
  * SO(3) projection in 4 groups of 8 tiles (pipelined against streaming):
    Frobenius-scaled Newton polar iteration (3 scaled + 2 plain), then the
    smallest eigenvalue of P = orth^T cam via a polynomial acos/sin closed form
    + 2 Newton polish steps, then reflection fix via the adjugate projector.
    Only Sqrt activation tables are used (no Ln/Exp/Arctan/Sin -> no per-group
    activation-table reloads).
"""

import numpy as np

import concourse.bacc as bacc
import concourse.mybir as mybir
from concourse.bass_types import AP
from concourse.tile import TileContext
from concourse import bass_utils

F32 = mybir.dt.float32
F16 = mybir.dt.float16
ALU = mybir.AluOpType
ACT = mybir.ActivationFunctionType

B_FULL = 32768
C = 256
E = 9
NCH = 18                              # 2304 = 18 chunks of 128
N_CORES = 8
P = 128
B_LOCAL = B_FULL // N_CORES           # 4096
TPC = B_LOCAL // P                    # 32 tiles of 128 batch items

import os
GROUPS = [int(t) for t in os.environ.get("KERNEL_GROUPS", "22,10").split(",")]
assert sum(GROUPS) == TPC
SCALED_ITERS = int(os.environ.get("KERNEL_SCALED", "2"))
PLAIN_ITERS = int(os.environ.get("KERNEL_PLAIN", "3"))                       # last plain iter is det-free (sign fold)
POLISH_ITERS = int(os.environ.get("KERNEL_POLISH", "0"))
XIN_BUFS = int(os.environ.get("KERNEL_XIN_BUFS", "12"))
QUAD = 4                              # tiles folded per PSUM accumulator

# acos(a) ~ sqrt(1-a)*(A0 + A1 a + A2 a^2 + A3 a^3), a in [0,1]  (A&S 4.4.45)
A0, A1, A2, A3 = 1.5707288, -0.2121144, 0.0742610, -0.0187293
# sin(phi) on [pi/6, pi/2], degree-4 chebfit (coeffs in monomial basis)
_xs = np.linspace(np.pi / 6, np.pi / 2, 512)
_cf = np.polynomial.chebyshev.chebfit(_xs, np.sin(_xs), 4)
S0, S1, S2, S3, S4 = [float(t) for t in np.polynomial.chebyshev.cheb2poly(_cf)]


def v(base: AP, off: int, *dims) -> AP:
    """Free-dim view of an SBUF tile AP: keep partition dim, set free dims."""
    return AP(base.tensor, base.offset + off,
              [list(base.ap[0])] + [[s, c] for (s, c) in dims])


def make_wm(W: np.ndarray) -> np.ndarray:
    """Masked split-fp16 W operand: wm[j, k, m] (m<9 hi, m>=9 lo) nonzero at
    m = ce % 9 with value W16[ce // 9], ce = 128j + k."""
    Wh = W.astype(np.float16)
    Wl = (W - Wh.astype(np.float32)).astype(np.float16)
    ce = np.arange(C * E)
    c, m = ce // E, ce % E
    wm = np.zeros((C * E, 18), np.float16)
    wm[ce, m] = Wh[c]
    wm[ce, 9 + m] = Wl[c]
    return np.ascontiguousarray(wm.reshape(NCH, P, 18))


def prepack_x(x: np.ndarray) -> np.ndarray:
    """[32768, 256, 3, 3] fp32 -> [8, 32, 128, 2, 18, 128] fp16.

    out[core, tt, k, s, j, p] = fp16-part-s of x[core*4096 + 32p + tt, ce]
    with ce = 128j + k. Per (core, tt): partition k holds 9216 contiguous
    bytes -> single fully-contiguous DMA per tile.
    """
    xr = np.asarray(x, dtype=np.float32).reshape(N_CORES, P, TPC, NCH, P)
    hi = xr.astype(np.float16)
    lo = (xr - hi.astype(np.float32)).astype(np.float16)
    out = np.empty((N_CORES, TPC, P, 2, NCH, P), np.float16)
    out[:, :, :, 0] = hi.transpose(0, 2, 4, 3, 1)
    out[:, :, :, 1] = lo.transpose(0, 2, 4, 3, 1)
    return out


def _so3_group(nc, wp, cam, T, g, y_flat, t0):
    """Project the T camera matrices in `cam` ([P, 9T], layout (t, e)) onto
    SO(3) and DMA the result to y rows {32p + t0 + i}.

    Two mostly-independent op chains (scheduler interleaves them on DVE):
      A) reflection block off M = cam^T cam: smallest eigenvalue lam3 = s3^2
         via polynomial acos/sin closed form + Newton polish, projector
         Z = I - f*adj(M - lam3 I)/tr  (f = 2*[det(cam) < 0])
      B) Frobenius-scaled Newton polar iteration (3 scaled + 1 plain +
         1 det-free sign-folded) -> orth
    Final: R = orth @ Z.  Chain A feeds chain B only through tr(M)/tr(cof M)
    (reused as the iter-0 Frobenius norms: ||cam||_F^2 = tr(M),
    ||cof cam||_F^2 = tr(cof M)).
    """
    vec = nc.vector
    act = nc.scalar
    NE = E * T

    def mk(n, name):
        return wp.tile([P, n], F32, name=f"{name}_{g}")

    # Newton-chain tiles
    Ya, Yb = mk(NE, 'Ya'), mk(NE, 'Yb')
    D = mk(36 * T, 'D')
    Cf, t1, t2, t3 = mk(NE, 'Cf'), mk(NE, 't1'), mk(NE, 't2'), mk(NE, 't3')
    td = mk(3 * T, 'td')
    det = mk(T, 'det')
    s1p, s2p = mk(T, 's1p'), mk(T, 's2p')
    u1, u2 = mk(T, 'u1'), mk(T, 'u2')
    fneg, sgnh = mk(T, 'fneg'), mk(T, 'sgnh')
    # M-chain tiles
    Mm, CM = mk(NE, 'Mm'), mk(NE, 'CM')
    DM = mk(36 * T, 'DM')
    m1, m2 = mk(NE, 'm1'), mk(NE, 'm2')
    tdm = mk(3 * T, 'tdm')
    cc2, cc1, cc0 = mk(T, 'cc2'), mk(T, 'cc1'), mk(T, 'cc0')
    qq, p26, pp = mk(T, 'qq'), mk(T, 'p26'), mk(T, 'pp')
    rr, lam3, w1 = mk(T, 'rr'), mk(T, 'lam3'), mk(T, 'w1')
    v1, v2, v3 = mk(T, 'v1'), mk(T, 'v2'), mk(T, 'v3')

    def mat(tile, off=0):
        return v(tile, off, (E, T), (3, 3), (1, 3))

    def flat(tile):
        return v(tile, 0, (1, NE))

    def row0(tile):
        return v(tile, 0, (E, T), (1, 3))

    def diag(tile):
        return v(tile, 0, (E, T), (4, 3))

    def pl(tile):
        return v(tile, 0, (1, T))

    def bc9(tile):
        return v(tile, 0, (1, T), (0, E))

    def bc3(tile):
        return v(tile, 0, (1, T), (0, 3))

    _consts = {}

    def cb(val):
        if val not in _consts:
            ct = wp.tile([P, 1], F32, name=f"const{g}_{len(_consts)}")
            vec.memset(ct[:], float(val))
            _consts[val] = ct[:]
        return _consts[val]

    tail = (t0 + T == TPC)

    def cofactor(Y, out, Dt, a, b):
        # out[i,j] = D[i+1,j+1]D[i+2,j+2] - D[i+1,j+2]D[i+2,j+1]
        # D = [[Y, Y], [Y, Y]] 6x6 tiling; copies split ACT/DVE, except in
        # the tail group where ACT round-trips would stall the serial chain.
        src = v(Y, 0, (E, T), (3, 3), (1, 3))
        for n, off in enumerate((0, 3, 18, 21)):
            eng_copy = act.copy if (n % 2 == 0 and not tail) else vec.tensor_copy
            eng_copy(v(Dt, off, (36, T), (6, 3), (1, 3)), src)

        def db(off):
            return v(Dt, off, (36, T), (6, 3), (1, 3))

        vec.tensor_tensor(mat(a), db(7), db(14), ALU.mult)
        vec.tensor_tensor(mat(b), db(8), db(13), ALU.mult)
        vec.tensor_tensor(mat(out), mat(a), mat(b), ALU.subtract)

    def det_of(Y, Cof, tdt, out):
        vec.tensor_tensor(v(tdt, 0, (3, T), (1, 3)), row0(Y), row0(Cof),
                          ALU.mult)
        vec.tensor_reduce(pl(out), v(tdt, 0, (3, T), (1, 3)),
                          mybir.AxisListType.X, ALU.add)

    # ================= chain A: reflection block off M = cam^T cam ========
    for k in range(3):
        a = v(cam, 3 * k, (E, T), (1, 3), (0, 3))
        b = v(cam, 3 * k, (E, T), (0, 3), (1, 3))
        if k == 0:
            vec.tensor_tensor(mat(Mm), a, b, ALU.mult)
        else:
            vec.tensor_tensor(mat(m1), a, b, ALU.mult)
            vec.tensor_tensor(mat(Mm), mat(Mm), mat(m1), ALU.add)
    cofactor(Mm, CM, DM, m1, m2)
    vec.tensor_reduce(pl(cc2), diag(Mm), mybir.AxisListType.X, ALU.add)
    vec.tensor_reduce(pl(cc1), diag(CM), mybir.AxisListType.X, ALU.add)
    det_of(Mm, CM, tdm, cc0)

    # smallest eigenvalue of M via trig closed form (lam3 = sigma3^2)
    vec.tensor_scalar_mul(pl(qq), pl(cc2), 1.0 / 3.0)
    vec.scalar_tensor_tensor(pl(p26), pl(cc2), 1.0 / 9.0, pl(cc2),
                             ALU.mult, ALU.mult)
    vec.scalar_tensor_tensor(pl(p26), pl(cc1), -1.0 / 3.0, pl(p26),
                             ALU.mult, ALU.add)
    vec.tensor_scalar(pl(p26), pl(p26), 0.0, None, ALU.max)
    act.activation(pl(pp), pl(p26), ACT.Sqrt, bias=cb(1e-30))  # sqrt(p26)
    # detB = ((2/3 c2) q - c1) q + c0
    vec.scalar_tensor_tensor(pl(rr), pl(cc2), 2.0 / 3.0, pl(qq),
                             ALU.mult, ALU.mult)
    vec.tensor_tensor(pl(rr), pl(rr), pl(cc1), ALU.subtract)
    vec.tensor_tensor(pl(rr), pl(rr), pl(qq), ALU.mult)
    vec.tensor_tensor(pl(rr), pl(rr), pl(cc0), ALU.add)
    # r = clamp(detB / (2 p^3 + eps))
    vec.tensor_tensor(pl(v1), pl(p26), pl(pp), ALU.mult)
    vec.tensor_scalar(pl(v1), pl(v1), 2.0, 1e-30, ALU.mult, ALU.add)
    vec.reciprocal(pl(v1), pl(v1))
    vec.tensor_tensor(pl(rr), pl(rr), pl(v1), ALU.mult)
    vec.tensor_scalar(pl(rr), pl(rr), -1.0, 1.0, ALU.max, ALU.min)
    # acos(|r|) via A&S poly * sqrt(1-|r|)
    vec.tensor_scalar_mul(pl(v1), pl(rr), -1.0)
    vec.tensor_tensor(pl(v1), pl(v1), pl(rr), ALU.max)         # |r|
    vec.tensor_scalar(pl(v2), pl(v1), A3, A2, ALU.mult, ALU.add)
    vec.tensor_tensor(pl(v2), pl(v2), pl(v1), ALU.mult)
    vec.tensor_scalar(pl(v2), pl(v2), A1, None, ALU.add)
    vec.tensor_tensor(pl(v2), pl(v2), pl(v1), ALU.mult)
    vec.tensor_scalar(pl(v2), pl(v2), A0, None, ALU.add)
    act.activation(pl(v3), pl(v1), ACT.Sqrt, scale=-1.0,
                   bias=cb(1.0))                               # sqrt(1-a)
    vec.tensor_tensor(pl(v2), pl(v2), pl(v3), ALU.mult)        # acos(|r|)
    # acos(r) = acos(|r|) + (r<0)*(pi - 2 acos(|r|))
    vec.tensor_scalar(pl(v1), pl(rr), 0.0, None, ALU.is_lt)
    vec.tensor_scalar(pl(v3), pl(v2), -2.0, np.pi, ALU.mult, ALU.add)
    vec.tensor_tensor(pl(v3), pl(v3), pl(v1), ALU.mult)
    vec.tensor_tensor(pl(v2), pl(v2), pl(v3), ALU.add)
    # phi = acos/3 + pi/6 ; sin(phi) degree-4 poly
    vec.tensor_scalar(pl(v2), pl(v2), 1.0 / 3.0, np.pi / 6.0,
                      ALU.mult, ALU.add)
    vec.tensor_scalar(pl(v3), pl(v2), S4, S3, ALU.mult, ALU.add)
    vec.tensor_tensor(pl(v3), pl(v3), pl(v2), ALU.mult)
    vec.tensor_scalar(pl(v3), pl(v3), S2, None, ALU.add)
    vec.tensor_tensor(pl(v3), pl(v3), pl(v2), ALU.mult)
    vec.tensor_scalar(pl(v3), pl(v3), S1, None, ALU.add)
    vec.tensor_tensor(pl(v3), pl(v3), pl(v2), ALU.mult)
    vec.tensor_scalar(pl(v3), pl(v3), S0, None, ALU.add)
    vec.tensor_tensor(pl(v3), pl(v3), pl(pp), ALU.mult)
    vec.scalar_tensor_tensor(pl(lam3), pl(v3), -2.0, pl(qq),
                             ALU.mult, ALU.add)                # lam3

    for _ in range(POLISH_ITERS):
        vec.tensor_tensor(pl(v1), pl(cc2), pl(lam3), ALU.subtract)
        vec.tensor_tensor(pl(v1), pl(v1), pl(lam3), ALU.mult)
        vec.tensor_tensor(pl(v1), pl(v1), pl(cc1), ALU.subtract)
        vec.tensor_tensor(pl(v1), pl(v1), pl(lam3), ALU.mult)
        vec.tensor_tensor(pl(v1), pl(v1), pl(cc0), ALU.add)
        vec.tensor_scalar(pl(v2), pl(lam3), -3.0, None, ALU.mult)
        vec.scalar_tensor_tensor(pl(v2), pl(cc2), 2.0, pl(v2),
                                 ALU.mult, ALU.add)
        vec.tensor_tensor(pl(v2), pl(v2), pl(lam3), ALU.mult)
        vec.tensor_tensor(pl(v2), pl(v2), pl(cc1), ALU.subtract)
        vec.tensor_scalar(pl(v2), pl(v2), -1e-20, None, ALU.add)
        vec.reciprocal(pl(v2), pl(v2))
        vec.tensor_tensor(pl(v1), pl(v1), pl(v2), ALU.mult)
        vec.tensor_tensor(pl(lam3), pl(lam3), pl(v1), ALU.subtract)

    # Nadj = CM + lam3*M + (lam3^2 - lam3*c2) I ; proj = Nadj / tr
    vec.tensor_tensor(pl(w1), pl(lam3), pl(cc2), ALU.mult)
    vec.tensor_tensor(pl(v1), pl(lam3), pl(lam3), ALU.mult)
    vec.tensor_tensor(pl(w1), pl(v1), pl(w1), ALU.subtract)
    vec.tensor_tensor(flat(m1), flat(Mm), bc9(lam3), ALU.mult)
    vec.tensor_tensor(flat(CM), flat(CM), flat(m1), ALU.add)
    vec.tensor_tensor(diag(CM), diag(CM), bc3(w1), ALU.add)
    vec.tensor_reduce(pl(v1), diag(CM), mybir.AxisListType.X, ALU.add)
    vec.tensor_scalar(pl(v1), pl(v1), 1e-30, None, ALU.add)
    vec.reciprocal(pl(v1), pl(v1))
    vec.tensor_tensor(flat(CM), flat(CM), bc9(v1), ALU.mult)   # proj

    # ================= chain B: Newton polar ==============================
    Y = cam
    other = [Ya, Yb]
    for it in range(SCALED_ITERS + PLAIN_ITERS):
        last = it == SCALED_ITERS + PLAIN_ITERS - 1
        cofactor(Y, Cf, D, t1, t2)
        Yn = other[it % 2]
        if last:
            # det -> sign(det0) at convergence: Yn = (Y + sgn*Cf)/2
            vec.tensor_tensor(flat(t2), flat(Cf), bc9(sgnh), ALU.mult)
            vec.tensor_scalar_mul(flat(t1), flat(Y), 0.5)
            vec.tensor_tensor(flat(Yn), flat(t1), flat(t2), ALU.add)
            Y = Yn
            continue
        det_of(Y, Cf, td, det)
        if it == 0:
            vec.tensor_scalar(pl(fneg), pl(det), 0.0, None, ALU.is_lt)
            vec.tensor_scalar(pl(sgnh), pl(fneg), -1.0, 0.5,
                              ALU.mult, ALU.add)               # +-0.5
        if it < SCALED_ITERS:
            if it == 0:
                # ||cam||_F^2 = tr(M) = cc2 ; ||cof cam||_F^2 = tr(CM) = cc1
                vec.reciprocal(pl(u1), pl(cc2))
                vec.tensor_tensor(pl(u1), pl(u1), pl(cc1), ALU.mult)
            else:
                vec.tensor_tensor(flat(t3), flat(Y), flat(Y), ALU.mult)
                vec.tensor_reduce(pl(u1), v(t3, 0, (E, T), (1, E)),
                                  mybir.AxisListType.X, ALU.add)
                vec.tensor_tensor(flat(t3), flat(Cf), flat(Cf), ALU.mult)
                vec.tensor_reduce(pl(u2), v(t3, 0, (E, T), (1, E)),
                                  mybir.AxisListType.X, ALU.add)
                vec.reciprocal(pl(u1), pl(u1))
                vec.tensor_tensor(pl(u1), pl(u1), pl(u2), ALU.mult)
            act.activation(pl(u1), pl(u1), ACT.Sqrt)           # (n2/n1)^1/2
            vec.tensor_scalar_mul(pl(u2), pl(det), -1.0)
            vec.tensor_tensor(pl(u2), pl(u2), pl(det), ALU.max)
            vec.tensor_scalar(pl(u2), pl(u2), 1e-35, None, ALU.add)
            vec.reciprocal(pl(u2), pl(u2))
            vec.tensor_tensor(pl(u1), pl(u1), pl(u2), ALU.mult)
            act.activation(pl(s1p), pl(u1), ACT.Sqrt)          # mu
            vec.tensor_tensor(pl(u1), pl(s1p), pl(det), ALU.mult)
            vec.reciprocal(pl(u1), pl(u1))
            vec.tensor_scalar_mul(pl(s2p), pl(u1), 0.5)        # 1/(2 mu det)
            vec.tensor_scalar_mul(pl(s1p), pl(s1p), 0.5)       # mu/2
            vec.tensor_tensor(flat(t1), flat(Y), bc9(s1p), ALU.mult)
            vec.tensor_tensor(flat(t2), flat(Cf), bc9(s2p), ALU.mult)
            vec.tensor_tensor(flat(Yn), flat(t1), flat(t2), ALU.add)
        else:
            vec.reciprocal(pl(s2p), pl(det))
            vec.tensor_scalar_mul(pl(s2p), pl(s2p), 0.5)
            vec.tensor_scalar_mul(flat(t1), flat(Y), 0.5)
            vec.tensor_tensor(flat(t2), flat(Cf), bc9(s2p), ALU.mult)
            vec.tensor_tensor(flat(Yn), flat(t1), flat(t2), ALU.add)
        Y = Yn
    orth = Y

    # ================= combine: R = orth @ (I - f*proj) ===================
    vec.tensor_scalar_mul(pl(v2), pl(fneg), -2.0)              # -f
    vec.tensor_tensor(flat(CM), flat(CM), bc9(v2), ALU.mult)
    vec.tensor_scalar(flat(CM), flat(CM), -2.0, 2.0, ALU.max, ALU.min)
    vec.tensor_scalar(diag(CM), diag(CM), 1.0, None, ALU.add)  # Z
    for k in range(3):
        a = v(orth, k, (E, T), (3, 3), (0, 3))
        b = v(CM, 3 * k, (E, T), (0, 3), (1, 3))
        if k == 0:
            vec.tensor_tensor(mat(m1), a, b, ALU.mult)
        else:
            vec.tensor_tensor(mat(m2), a, b, ALU.mult)
            vec.tensor_tensor(mat(m1), mat(m1), mat(m2), ALU.add)

    # earlier groups store via SWDGE (spread across engines, HWDGE x-load
    # rings untouched); the final group streams after the x-load is done, so
    # use the lower-latency HWDGE ring for it
    if t0 + T == TPC:
        nc.scalar.dma_start(out=y_flat[:, E * t0: E * (t0 + T)], in_=flat(m1))
    else:
        nc.gpsimd.dma_start(out=y_flat[:, E * t0: E * (t0 + T)], in_=flat(m1))


def _emit(nc, tc, x_ap, wm_ap, y_ap):
    vec = nc.vector
    y_flat = y_ap.rearrange("b h w -> b (h w)").rearrange(
        "(p t) e -> p (t e)", p=P)

    with tc.tile_pool(name="xin", bufs=XIN_BUFS) as xpool, \
         tc.tile_pool(name="pcp", bufs=8, space="PSUM") as pcp, \
         tc.tile_pool(name="wk", bufs=1) as wp:
        # masked W -> SBUF [k, (j, n)]
        wm_sb = wp.tile([P, NCH * 18], F16)
        nc.sync.dma_start(
            out=wm_sb[:],
            in_=AP(wm_ap.tensor, 0, [[18, P], [P * 18, NCH], [1, 18]]))

        t0 = 0
        for g, T in enumerate(GROUPS):
            cam = wp.tile([P, E * T], F32, name=f"cam{g}")
            i = 0
            while i < T:
                blk = min(QUAD, T - i)
                pc = pcp.tile([P, 18 * blk], F32, tag="pc",
                              name=f"pc{t0 + i}")
                for b in range(blk):
                    tt = t0 + i + b
                    xt = xpool.tile([P, 2 * NCH * P], F16, tag="xt",
                                    name=f"xt{tt}")
                    nc.sync.dma_start(
                        out=xt[:],
                        in_=AP(x_ap.tensor, tt * P * 2 * NCH * P,
                               [[2 * NCH * P, P], [1, 2 * NCH * P]]))
                    # cols 0-8: xh*Wh (+ xl*Wh), cols 9-17: xh*Wl
                    for j in range(NCH):
                        nc.tensor.matmul(pc[:, 18 * b:18 * b + 18],
                                         xt[:, j * P:(j + 1) * P],
                                         v(wm_sb, 18 * j, (1, 18)),
                                         start=(j == 0), stop=False)
                    for j in range(NCH):
                        nc.tensor.matmul(pc[:, 18 * b:18 * b + 9],
                                         xt[:, (NCH + j) * P:(NCH + j + 1) * P],
                                         v(wm_sb, 18 * j, (1, 9)),
                                         start=False, stop=(j == NCH - 1))
                # cam[:, (i+b)*9 + e] = pc[18b + e] + pc[18b + 9 + e]
                # one strided PSUM reduce folds hi+lo for the whole quad
                vec.tensor_reduce(v(cam, E * i, (9, blk), (1, 9)),
                                  v(pc[:], 0, (18, blk), (1, 9), (9, 2)),
                                  mybir.AxisListType.X, ALU.add)
                i += blk
            _so3_group(nc, wp, cam, T, g, y_flat, t0)
            t0 += T


def build():
    nc = bacc.Bacc("TRN2", target_bir_lowering=False, debug=False)
    x = nc.dram_tensor("x", [TPC, P, 2 * NCH * P], F16, kind="ExternalInput")
    wm = nc.dram_tensor("wm", [NCH, P, 18], F16, kind="ExternalInput")
    y = nc.dram_tensor("y", [B_LOCAL, 3, 3], F32, kind="ExternalOutput")
    with TileContext(nc) as tc:
        _emit(nc, tc, x.ap(), wm.ap(), y.ap())
    nc.compile()
    return nc


_NC_CACHE = {}


def prepare_inputs(x: np.ndarray, W: np.ndarray):
    xt = prepack_x(x)
    wm = make_wm(np.asarray(W, dtype=np.float32))
    return [{"x": xt[i], "wm": wm} for i in range(N_CORES)]


def kernel(x: np.ndarray, W: np.ndarray) -> np.ndarray:
    assert x.shape == (B_FULL, C, 3, 3) and W.shape == (C,)
    if "nc" not in _NC_CACHE:
        _NC_CACHE["nc"] = build()
    nc = _NC_CACHE["nc"]
    in_maps = prepare_inputs(x, W)
    res = bass_utils.run_bass_kernel_spmd(nc, in_maps,
                                          core_ids=list(range(N_CORES)))
    return np.concatenate([r["y"] for r in res.results], axis=0)


if __name__ == "__main__":
    rng = np.random.default_rng(0)
    x = rng.standard_normal((B_FULL, C, 3, 3), dtype=np.float32)
    W = (rng.standard_normal(C, dtype=np.float32) / np.sqrt(C)).astype(np.float32)
    out = kernel(x=x, W=W)
    print(out.shape, out.dtype)


# revision 18
# speedup vs baseline: 1.0565x; 1.0565x over previous
"""Trainium2 Bass kernel for nn_CameraEstimator.

For each batch item b:
    camera[b] = einsum('chw,c->hw', x[b], W)          (C=256 contraction)
    out[b]    = nearest-rotation(camera[b])           (SVD u@vh + det reflection fix)

Pipeline (per core, B split 8 ways):
  * Host packs x into a transposed split-fp16 layout: for each 128-batch tile,
    partition k holds the (c,e)-flattened entries ce = 128j + k (j = 0..17) for
    128 batch items, as fp16 hi and fp16 lo (hi = fp16(x), lo = fp16(x - hi)).
    Same total HBM bytes as fp32 x; enables direct PE contraction with zero
    on-device transposes/casts.
  * Device: per tile, 36 accumulating matmuls (18 hi + 18 lo chunks) against a
    masked-W operand -> camera [128, 9hi+9lo] in PSUM; one DVE add folds them.
  * SO(3) projection in 4 groups of 8 tiles (pipelined against streaming):
    Frobenius-scaled Newton polar iteration (3 scaled + 2 plain), then the
    smallest eigenvalue of P = orth^T cam via a polynomial acos/sin closed form
    + 2 Newton polish steps, then reflection fix via the adjugate projector.
    Only Sqrt activation tables are used (no Ln/Exp/Arctan/Sin -> no per-group
    activation-table reloads).
"""

import numpy as np

import concourse.bacc as bacc
import concourse.mybir as mybir
from concourse.bass_types import AP
from concourse.tile import TileContext
from concourse import bass_utils

F32 = mybir.dt.float32
F16 = mybir.dt.float16
ALU = mybir.AluOpType
ACT = mybir.ActivationFunctionType

B_FULL = 32768
C = 256
E = 9
NCH = 18                              # 2304 = 18 chunks of 128
N_CORES = 8
P = 128
B_LOCAL = B_FULL // N_CORES           # 4096
TPC = B_LOCAL // P                    # 32 tiles of 128 batch items

import os
GROUPS = [int(t) for t in os.environ.get("KERNEL_GROUPS", "22,10").split(",")]
assert sum(GROUPS) == TPC
SCALED_ITERS = int(os.environ.get("KERNEL_SCALED", "2"))
PLAIN_ITERS = int(os.environ.get("KERNEL_PLAIN", "3"))                       # last plain iter is det-free (sign fold)
POLISH_ITERS = int(os.environ.get("KERNEL_POLISH", "0"))
XIN_BUFS = int(os.environ.get("KERNEL_XIN_BUFS", "12"))
QUAD = 4                              # tiles folded per PSUM accumulator

# acos(a) ~ sqrt(1-a)*(A0 + A1 a + A2 a^2 + A3 a^3), a in [0,1]  (A&S 4.4.45)
A0, A1, A2, A3 = 1.5707288, -0.2121144, 0.0742610, -0.0187293
# sin(phi) on [pi/6, pi/2], degree-4 chebfit (coeffs in monomial basis)
_xs = np.linspace(np.pi / 6, np.pi / 2, 512)
_cf = np.polynomial.chebyshev.chebfit(_xs, np.sin(_xs), 4)
S0, S1, S2, S3, S4 = [float(t) for t in np.polynomial.chebyshev.cheb2poly(_cf)]


def v(base: AP, off: int, *dims) -> AP:
    """Free-dim view of an SBUF tile AP: keep partition dim, set free dims."""
    return AP(base.tensor, base.offset + off,
              [list(base.ap[0])] + [[s, c] for (s, c) in dims])


def make_wm(W: np.ndarray) -> np.ndarray:
    """Masked split-fp16 W operand: wm[j, k, m] (m<9 hi, m>=9 lo) nonzero at
    m = ce % 9 with value W16[ce // 9], ce = 128j + k."""
    Wh = W.astype(np.float16)
    Wl = (W - Wh.astype(np.float32)).astype(np.float16)
    ce = np.arange(C * E)
    c, m = ce // E, ce % E
    wm = np.zeros((C * E, 18), np.float16)
    wm[ce, m] = Wh[c]
    wm[ce, 9 + m] = Wl[c]
    return np.ascontiguousarray(wm.reshape(NCH, P, 18))


def prepack_x(x: np.ndarray) -> np.ndarray:
    """[32768, 256, 3, 3] fp32 -> [8, 32, 128, 2, 18, 128] fp16.

    out[core, tt, k, s, j, p] = fp16-part-s of x[core*4096 + 32p + tt, ce]
    with ce = 128j + k. Per (core, tt): partition k holds 9216 contiguous
    bytes -> single fully-contiguous DMA per tile.
    """
    xr = np.asarray(x, dtype=np.float32).reshape(N_CORES, P, TPC, NCH, P)
    hi = xr.astype(np.float16)
    lo = (xr - hi.astype(np.float32)).astype(np.float16)
    out = np.empty((N_CORES, TPC, P, 2, NCH, P), np.float16)
    out[:, :, :, 0] = hi.transpose(0, 2, 4, 3, 1)
    out[:, :, :, 1] = lo.transpose(0, 2, 4, 3, 1)
    return out


def _so3_group(nc, wp, cam, T, g, y_flat, t0):
    """Project the T camera matrices in `cam` ([P, 9T], layout (t, e)) onto
    SO(3) and DMA the result to y rows {32p + t0 + i}.

    Two mostly-independent op chains (scheduler interleaves them on DVE):
      A) reflection block off M = cam^T cam: smallest eigenvalue lam3 = s3^2
         via polynomial acos/sin closed form + Newton polish, projector
         Z = I - f*adj(M - lam3 I)/tr  (f = 2*[det(cam) < 0])
      B) Frobenius-scaled Newton polar iteration (3 scaled + 1 plain +
         1 det-free sign-folded) -> orth
    Final: R = orth @ Z.  Chain A feeds chain B only through tr(M)/tr(cof M)
    (reused as the iter-0 Frobenius norms: ||cam||_F^2 = tr(M),
    ||cof cam||_F^2 = tr(cof M)).
    """
    vec = nc.vector
    act = nc.scalar
    NE = E * T

    def mk(n, name):
        return wp.tile([P, n], F32, name=f"{name}_{g}")

    # Newton-chain tiles
    Ya, Yb = mk(NE, 'Ya'), mk(NE, 'Yb')
    D = mk(36 * T, 'D')
    Cf, t1, t2, t3 = mk(NE, 'Cf'), mk(NE, 't1'), mk(NE, 't2'), mk(NE, 't3')
    td = mk(3 * T, 'td')
    det = mk(T, 'det')
    s1p, s2p = mk(T, 's1p'), mk(T, 's2p')
    u1, u2 = mk(T, 'u1'), mk(T, 'u2')
    fneg, sgnh = mk(T, 'fneg'), mk(T, 'sgnh')
    # M-chain tiles
    Mm, CM = mk(NE, 'Mm'), mk(NE, 'CM')
    DM = mk(36 * T, 'DM')
    m1, m2 = mk(NE, 'm1'), mk(NE, 'm2')
    tdm = mk(3 * T, 'tdm')
    cc2, cc1, cc0 = mk(T, 'cc2'), mk(T, 'cc1'), mk(T, 'cc0')
    qq, p26, pp = mk(T, 'qq'), mk(T, 'p26'), mk(T, 'pp')
    rr, lam3, w1 = mk(T, 'rr'), mk(T, 'lam3'), mk(T, 'w1')
    v1, v2, v3 = mk(T, 'v1'), mk(T, 'v2'), mk(T, 'v3')

    def mat(tile, off=0):
        return v(tile, off, (E, T), (3, 3), (1, 3))

    def flat(tile):
        return v(tile, 0, (1, NE))

    def row0(tile):
        return v(tile, 0, (E, T), (1, 3))

    def diag(tile):
        return v(tile, 0, (E, T), (4, 3))

    def pl(tile):
        return v(tile, 0, (1, T))

    def bc9(tile):
        return v(tile, 0, (1, T), (0, E))

    def bc3(tile):
        return v(tile, 0, (1, T), (0, 3))

    _consts = {}

    def cb(val):
        if val not in _consts:
            ct = wp.tile([P, 1], F32, name=f"const{g}_{len(_consts)}")
            vec.memset(ct[:], float(val))
            _consts[val] = ct[:]
        return _consts[val]

    def cofactor(Y, out, Dt, a, b):
        # out[i,j] = D[i+1,j+1]D[i+2,j+2] - D[i+1,j+2]D[i+2,j+1]
        # D = [[Y, Y], [Y, Y]] 6x6 tiling; copies split ACT/DVE.
        src = v(Y, 0, (E, T), (3, 3), (1, 3))
        for n, off in enumerate((0, 3, 18, 21)):
            eng_copy = act.copy if n % 2 == 0 else vec.tensor_copy
            eng_copy(v(Dt, off, (36, T), (6, 3), (1, 3)), src)

        def db(off):
            return v(Dt, off, (36, T), (6, 3), (1, 3))

        vec.tensor_tensor(mat(a), db(7), db(14), ALU.mult)
        vec.tensor_tensor(mat(b), db(8), db(13), ALU.mult)
        vec.tensor_tensor(mat(out), mat(a), mat(b), ALU.subtract)

    def det_of(Y, Cof, tdt, out):
        vec.tensor_tensor(v(tdt, 0, (3, T), (1, 3)), row0(Y), row0(Cof),
                          ALU.mult)
        vec.tensor_reduce(pl(out), v(tdt, 0, (3, T), (1, 3)),
                          mybir.AxisListType.X, ALU.add)

    # ================= chain A: reflection block off M = cam^T cam ========
    for k in range(3):
        a = v(cam, 3 * k, (E, T), (1, 3), (0, 3))
        b = v(cam, 3 * k, (E, T), (0, 3), (1, 3))
        if k == 0:
            vec.tensor_tensor(mat(Mm), a, b, ALU.mult)
        else:
            vec.tensor_tensor(mat(m1), a, b, ALU.mult)
            vec.tensor_tensor(mat(Mm), mat(Mm), mat(m1), ALU.add)
    cofactor(Mm, CM, DM, m1, m2)
    vec.tensor_reduce(pl(cc2), diag(Mm), mybir.AxisListType.X, ALU.add)
    vec.tensor_reduce(pl(cc1), diag(CM), mybir.AxisListType.X, ALU.add)
    det_of(Mm, CM, tdm, cc0)

    # smallest eigenvalue of M via trig closed form (lam3 = sigma3^2)
    vec.tensor_scalar_mul(pl(qq), pl(cc2), 1.0 / 3.0)
    vec.scalar_tensor_tensor(pl(p26), pl(cc2), 1.0 / 9.0, pl(cc2),
                             ALU.mult, ALU.mult)
    vec.scalar_tensor_tensor(pl(p26), pl(cc1), -1.0 / 3.0, pl(p26),
                             ALU.mult, ALU.add)
    vec.tensor_scalar(pl(p26), pl(p26), 0.0, None, ALU.max)
    act.activation(pl(pp), pl(p26), ACT.Sqrt, bias=cb(1e-30))  # sqrt(p26)
    # detB = ((2/3 c2) q - c1) q + c0
    vec.scalar_tensor_tensor(pl(rr), pl(cc2), 2.0 / 3.0, pl(qq),
                             ALU.mult, ALU.mult)
    vec.tensor_tensor(pl(rr), pl(rr), pl(cc1), ALU.subtract)
    vec.tensor_tensor(pl(rr), pl(rr), pl(qq), ALU.mult)
    vec.tensor_tensor(pl(rr), pl(rr), pl(cc0), ALU.add)
    # r = clamp(detB / (2 p^3 + eps))
    vec.tensor_tensor(pl(v1), pl(p26), pl(pp), ALU.mult)
    vec.tensor_scalar(pl(v1), pl(v1), 2.0, 1e-30, ALU.mult, ALU.add)
    vec.reciprocal(pl(v1), pl(v1))
    vec.tensor_tensor(pl(rr), pl(rr), pl(v1), ALU.mult)
    vec.tensor_scalar(pl(rr), pl(rr), -1.0, 1.0, ALU.max, ALU.min)
    # acos(|r|) via A&S poly * sqrt(1-|r|)
    vec.tensor_scalar_mul(pl(v1), pl(rr), -1.0)
    vec.tensor_tensor(pl(v1), pl(v1), pl(rr), ALU.max)         # |r|
    vec.tensor_scalar(pl(v2), pl(v1), A3, A2, ALU.mult, ALU.add)
    vec.tensor_tensor(pl(v2), pl(v2), pl(v1), ALU.mult)
    vec.tensor_scalar(pl(v2), pl(v2), A1, None, ALU.add)
    vec.tensor_tensor(pl(v2), pl(v2), pl(v1), ALU.mult)
    vec.tensor_scalar(pl(v2), pl(v2), A0, None, ALU.add)
    act.activation(pl(v3), pl(v1), ACT.Sqrt, scale=-1.0,
                   bias=cb(1.0))                               # sqrt(1-a)
    vec.tensor_tensor(pl(v2), pl(v2), pl(v3), ALU.mult)        # acos(|r|)
    # acos(r) = acos(|r|) + (r<0)*(pi - 2 acos(|r|))
    vec.tensor_scalar(pl(v1), pl(rr), 0.0, None, ALU.is_lt)
    vec.tensor_scalar(pl(v3), pl(v2), -2.0, np.pi, ALU.mult, ALU.add)
    vec.tensor_tensor(pl(v3), pl(v3), pl(v1), ALU.mult)
    vec.tensor_tensor(pl(v2), pl(v2), pl(v3), ALU.add)
    # phi = acos/3 + pi/6 ; sin(phi) degree-4 poly
    vec.tensor_scalar(pl(v2), pl(v2), 1.0 / 3.0, np.pi / 6.0,
                      ALU.mult, ALU.add)
    vec.tensor_scalar(pl(v3), pl(v2), S4, S3, ALU.mult, ALU.add)
    vec.tensor_tensor(pl(v3), pl(v3), pl(v2), ALU.mult)
    vec.tensor_scalar(pl(v3), pl(v3), S2, None, ALU.add)
    vec.tensor_tensor(pl(v3), pl(v3), pl(v2), ALU.mult)
    vec.tensor_scalar(pl(v3), pl(v3), S1, None, ALU.add)
    vec.tensor_tensor(pl(v3), pl(v3), pl(v2), ALU.mult)
    vec.tensor_scalar(pl(v3), pl(v3), S0, None, ALU.add)
    vec.tensor_tensor(pl(v3), pl(v3), pl(pp), ALU.mult)
    vec.scalar_tensor_tensor(pl(lam3), pl(v3), -2.0, pl(qq),
                             ALU.mult, ALU.add)                # lam3

    for _ in range(POLISH_ITERS):
        vec.tensor_tensor(pl(v1), pl(cc2), pl(lam3), ALU.subtract)
        vec.tensor_tensor(pl(v1), pl(v1), pl(lam3), ALU.mult)
        vec.tensor_tensor(pl(v1), pl(v1), pl(cc1), ALU.subtract)
        vec.tensor_tensor(pl(v1), pl(v1), pl(lam3), ALU.mult)
        vec.tensor_tensor(pl(v1), pl(v1), pl(cc0), ALU.add)
        vec.tensor_scalar(pl(v2), pl(lam3), -3.0, None, ALU.mult)
        vec.scalar_tensor_tensor(pl(v2), pl(cc2), 2.0, pl(v2),
                                 ALU.mult, ALU.add)
        vec.tensor_tensor(pl(v2), pl(v2), pl(lam3), ALU.mult)
        vec.tensor_tensor(pl(v2), pl(v2), pl(cc1), ALU.subtract)
        vec.tensor_scalar(pl(v2), pl(v2), -1e-20, None, ALU.add)
        vec.reciprocal(pl(v2), pl(v2))
        vec.tensor_tensor(pl(v1), pl(v1), pl(v2), ALU.mult)
        vec.tensor_tensor(pl(lam3), pl(lam3), pl(v1), ALU.subtract)

    # Nadj = CM + lam3*M + (lam3^2 - lam3*c2) I ; proj = Nadj / tr
    vec.tensor_tensor(pl(w1), pl(lam3), pl(cc2), ALU.mult)
    vec.tensor_tensor(pl(v1), pl(lam3), pl(lam3), ALU.mult)
    vec.tensor_tensor(pl(w1), pl(v1), pl(w1), ALU.subtract)
    vec.tensor_tensor(flat(m1), flat(Mm), bc9(lam3), ALU.mult)
    vec.tensor_tensor(flat(CM), flat(CM), flat(m1), ALU.add)
    vec.tensor_tensor(diag(CM), diag(CM), bc3(w1), ALU.add)
    vec.tensor_reduce(pl(v1), diag(CM), mybir.AxisListType.X, ALU.add)
    vec.tensor_scalar(pl(v1), pl(v1), 1e-30, None, ALU.add)
    vec.reciprocal(pl(v1), pl(v1))
    vec.tensor_tensor(flat(CM), flat(CM), bc9(v1), ALU.mult)   # proj

    # ================= chain B: Newton polar ==============================
    Y = cam
    other = [Ya, Yb]
    for it in range(SCALED_ITERS + PLAIN_ITERS):
        last = it == SCALED_ITERS + PLAIN_ITERS - 1
        cofactor(Y, Cf, D, t1, t2)
        Yn = other[it % 2]
        if last:
            # det -> sign(det0) at convergence: Yn = (Y + sgn*Cf)/2
            vec.tensor_tensor(flat(t2), flat(Cf), bc9(sgnh), ALU.mult)
            vec.tensor_scalar_mul(flat(t1), flat(Y), 0.5)
            vec.tensor_tensor(flat(Yn), flat(t1), flat(t2), ALU.add)
            Y = Yn
            continue
        det_of(Y, Cf, td, det)
        if it == 0:
            vec.tensor_scalar(pl(fneg), pl(det), 0.0, None, ALU.is_lt)
            vec.tensor_scalar(pl(sgnh), pl(fneg), -1.0, 0.5,
                              ALU.mult, ALU.add)               # +-0.5
        if it < SCALED_ITERS:
            if it == 0:
                # ||cam||_F^2 = tr(M) = cc2 ; ||cof cam||_F^2 = tr(CM) = cc1
                vec.reciprocal(pl(u1), pl(cc2))
                vec.tensor_tensor(pl(u1), pl(u1), pl(cc1), ALU.mult)
            else:
                vec.tensor_tensor(flat(t3), flat(Y), flat(Y), ALU.mult)
                vec.tensor_reduce(pl(u1), v(t3, 0, (E, T), (1, E)),
                                  mybir.AxisListType.X, ALU.add)
                vec.tensor_tensor(flat(t3), flat(Cf), flat(Cf), ALU.mult)
                vec.tensor_reduce(pl(u2), v(t3, 0, (E, T), (1, E)),
                                  mybir.AxisListType.X, ALU.add)
                vec.reciprocal(pl(u1), pl(u1))
                vec.tensor_tensor(pl(u1), pl(u1), pl(u2), ALU.mult)
            act.activation(pl(u1), pl(u1), ACT.Sqrt)           # (n2/n1)^1/2
            vec.tensor_scalar_mul(pl(u2), pl(det), -1.0)
            vec.tensor_tensor(pl(u2), pl(u2), pl(det), ALU.max)
            vec.tensor_scalar(pl(u2), pl(u2), 1e-35, None, ALU.add)
            vec.reciprocal(pl(u2), pl(u2))
            vec.tensor_tensor(pl(u1), pl(u1), pl(u2), ALU.mult)
            act.activation(pl(s1p), pl(u1), ACT.Sqrt)          # mu
            vec.tensor_tensor(pl(u1), pl(s1p), pl(det), ALU.mult)
            vec.reciprocal(pl(u1), pl(u1))
            vec.tensor_scalar_mul(pl(s2p), pl(u1), 0.5)        # 1/(2 mu det)
            vec.tensor_scalar_mul(pl(s1p), pl(s1p), 0.5)       # mu/2
            vec.tensor_tensor(flat(t1), flat(Y), bc9(s1p), ALU.mult)
            vec.tensor_tensor(flat(t2), flat(Cf), bc9(s2p), ALU.mult)
            vec.tensor_tensor(flat(Yn), flat(t1), flat(t2), ALU.add)
        else:
            vec.reciprocal(pl(s2p), pl(det))
            vec.tensor_scalar_mul(pl(s2p), pl(s2p), 0.5)
            vec.tensor_scalar_mul(flat(t1), flat(Y), 0.5)
            vec.tensor_tensor(flat(t2), flat(Cf), bc9(s2p), ALU.mult)
            vec.tensor_tensor(flat(Yn), flat(t1), flat(t2), ALU.add)
        Y = Yn
    orth = Y

    # ================= combine: R = orth @ (I - f*proj) ===================
    vec.tensor_scalar_mul(pl(v2), pl(fneg), -2.0)              # -f
    vec.tensor_tensor(flat(CM), flat(CM), bc9(v2), ALU.mult)
    vec.tensor_scalar(flat(CM), flat(CM), -2.0, 2.0, ALU.max, ALU.min)
    vec.tensor_scalar(diag(CM), diag(CM), 1.0, None, ALU.add)  # Z
    for k in range(3):
        a = v(orth, k, (E, T), (3, 3), (0, 3))
        b = v(CM, 3 * k, (E, T), (0, 3), (1, 3))
        if k == 0:
            vec.tensor_tensor(mat(m1), a, b, ALU.mult)
        else:
            vec.tensor_tensor(mat(m2), a, b, ALU.mult)
            vec.tensor_tensor(mat(m1), mat(m1), mat(m2), ALU.add)

    # SWDGE (gpsimd) path: spreads the store across all 16 SDMA engines and
    # keeps the HWDGE x-load rings unblocked
    nc.gpsimd.dma_start(out=y_flat[:, E * t0: E * (t0 + T)], in_=flat(m1))


def _emit(nc, tc, x_ap, wm_ap, y_ap):
    vec = nc.vector
    y_flat = y_ap.rearrange("b h w -> b (h w)").rearrange(
        "(p t) e -> p (t e)", p=P)

    with tc.tile_pool(name="xin", bufs=XIN_BUFS) as xpool, \
         tc.tile_pool(name="pcp", bufs=8, space="PSUM") as pcp, \
         tc.tile_pool(name="wk", bufs=1) as wp:
        # masked W -> SBUF [k, (j, n)]
        wm_sb = wp.tile([P, NCH * 18], F16)
        nc.sync.dma_start(
            out=wm_sb[:],
            in_=AP(wm_ap.tensor, 0, [[18, P], [P * 18, NCH], [1, 18]]))

        t0 = 0
        for g, T in enumerate(GROUPS):
            cam = wp.tile([P, E * T], F32, name=f"cam{g}")
            i = 0
            while i < T:
                blk = min(QUAD, T - i)
                pc = pcp.tile([P, 18 * blk], F32, tag="pc",
                              name=f"pc{t0 + i}")
                for b in range(blk):
                    tt = t0 + i + b
                    xt = xpool.tile([P, 2 * NCH * P], F16, tag="xt",
                                    name=f"xt{tt}")
                    nc.sync.dma_start(
                        out=xt[:],
                        in_=AP(x_ap.tensor, tt * P * 2 * NCH * P,
                               [[2 * NCH * P, P], [1, 2 * NCH * P]]))
                    # cols 0-8: xh*Wh (+ xl*Wh), cols 9-17: xh*Wl
                    for j in range(NCH):
                        nc.tensor.matmul(pc[:, 18 * b:18 * b + 18],
                                         xt[:, j * P:(j + 1) * P],
                                         v(wm_sb, 18 * j, (1, 18)),
                                         start=(j == 0), stop=False)
                    for j in range(NCH):
                        nc.tensor.matmul(pc[:, 18 * b:18 * b + 9],
                                         xt[:, (NCH + j) * P:(NCH + j + 1) * P],
                                         v(wm_sb, 18 * j, (1, 9)),
                                         start=False, stop=(j == NCH - 1))
                # cam[:, (i+b)*9 + e] = pc[18b + e] + pc[18b + 9 + e]
                # one strided PSUM reduce folds hi+lo for the whole quad
                vec.tensor_reduce(v(cam, E * i, (9, blk), (1, 9)),
                                  v(pc[:], 0, (18, blk), (1, 9), (9, 2)),
                                  mybir.AxisListType.X, ALU.add)
                i += blk
            _so3_group(nc, wp, cam, T, g, y_flat, t0)
            t0 += T


def build():
    nc = bacc.Bacc("TRN2", target_bir_lowering=False, debug=False)
    x = nc.dram_tensor("x", [TPC, P, 2 * NCH * P], F16, kind="ExternalInput")
    wm = nc.dram_tensor("wm", [NCH, P, 18], F16, kind="ExternalInput")
    y = nc.dram_tensor("y", [B_LOCAL, 3, 3], F32, kind="ExternalOutput")
    with TileContext(nc) as tc:
        _emit(nc, tc, x.ap(), wm.ap(), y.ap())
    nc.compile()
    return nc


_NC_CACHE = {}


def prepare_inputs(x: np.ndarray, W: np.ndarray):
    xt = prepack_x(x)
    wm = make_wm(np.asarray(W, dtype=np.float32))
    return [{"x": xt[i], "wm": wm} for i in range(N_CORES)]


def kernel(x: np.ndarray, W: np.ndarray) -> np.ndarray:
    assert x.shape == (B_FULL, C, 3, 3) and W.shape == (C,)
    if "nc" not in _NC_CACHE:
        _NC_CACHE["nc"] = build()
    nc = _NC_CACHE["nc"]
    in_maps = prepare_inputs(x, W)
    res = bass_utils.run_bass_kernel_spmd(nc, in_maps,
                                          core_ids=list(range(N_CORES)))
    return np.concatenate([r["y"] for r in res.results], axis=0)


if __name__ == "__main__":
    rng = np.random.default_rng(0)
    x = rng.standard_normal((B_FULL, C, 3, 3), dtype=np.float32)
    W = (rng.standard_normal(C, dtype=np.float32) / np.sqrt(C)).astype(np.float32)
    out = kernel(x=x, W=W)
    print(out.shape, out.dtype)
